# revision 24
# baseline (speedup 1.0000x reference)
"""Causal multi-head attention (b=2, h=32, s=2048, d=128, fp32) on 8 TRN2 NeuronCores.

Sharding: the 64 (batch, head) pairs are split 8-per-core (tensor parallel over
heads); each core runs an identical Bass/Tile kernel on its own heads.

Per-core kernel, S^T formulation:
  Q^T/K^T ship as bf16 (halves their DMA; bf16 matmuls run 1 cycle/row at any
  width).  The 40 causal S^T tile-segments per head are packed (widest-fit-
  decreasing, PSUM-bank-aligned) into groups alternating 1536/1024 columns
  (tags sA/sB: 3+2 banks), so each exp instruction covers ~1280 columns and
  the scalar engine's ~285ns/instruction overhead is amortized over ~109
  instructions.  P^T = exp(S^T/sqrt(d)) is written as f32r (bf16 activation
  output measures ~25% slower per column), no max-subtraction needed.
  Diagonal-tile triangles are zeroed by DVE multiplies with a constant 0/1
  triangle, two triangles fused per op via a strided access pattern.
  ctx^T[d,q] accumulates in PSUM via V-stationary f32r matmuls (2 x 1 bank).
  The softmax denominator l[q] is accumulated per 128-k-partition in two
  independent SBUF accumulators - one summed by the DVE, one by the gpsimd
  engine (decoupled serial chains, no cross-engine hops) - then both are
  partition-reduced by ones-stationary PE matmuls into a single PSUM bank
  (3+2+1+2 = 8 banks total).  Epilogue: recip (DVE) + ctx*recip (DVE), DMA
  out as bf16.  Cleanups/epilogues trail the main loop by 1-2 groups so no
  engine head-of-line blocks another.
"""
import math
import sys

if '/opt/trn_rl_repo' not in sys.path:
    sys.path.insert(0, '/opt/trn_rl_repo')

import numpy as np
import ml_dtypes

import concourse.bass as bass
import concourse.tile as tile
from concourse import mybir, bacc
from concourse.bass_utils import run_bass_kernel_spmd

F32 = mybir.dt.float32
F32R = mybir.dt.float32r
BF16 = mybir.dt.bfloat16
EXP = mybir.ActivationFunctionType.Exp
MULT = mybir.AluOpType.mult
ADD = mybir.AluOpType.add

B, H, S, D = 2, 32, 2048, 128
N_CORES = 8
HPC = (B * H) // N_CORES     # (b,h) pairs per core
QB = 512                     # q-block width
NQB = S // QB
NKT = S // 128               # k-tiles per head
SCALE = 1.0 / math.sqrt(D)


def _plan(n_heads, gp_frac=0.31, gp_min_cols=2944):
    """Pack all (head, q-block, k-tile) S^T segments into PSUM groups with
    capacities alternating 1536/1024 columns (widest-fit-decreasing inside a
    block; 512-wide segs front-fill banks, narrower back-fill).  Each segment
    gets: pos (column in group), eng ('dve'|'gp') and role
    ('pair1'|'pair2'|'copy'|'add') for the l accumulation."""
    groups = []
    cur, banks = [], [0] * 3
    blocks = {}

    def new_group():
        nonlocal cur, banks
        nbanks = 3 if len(groups) % 2 == 0 else 2
        cur, banks = [], [0] * nbanks

    new_group()
    for h in range(n_heads):
        for j in range(NQB):
            rem = []
            for t in range(4 * j + 4):    # causal: k-tiles 0..4j+3
                o = max(t - 4 * j, 0) * 128
                rem.append(dict(h=h, j=j, t=t, w=QB - o, o=o,
                                first=False, last=False))
            blk = blocks[(h, j)] = []
            first = True
            while rem:
                placed = None
                for sg in sorted(rem, key=lambda s: -s["w"]):
                    w = sg["w"]
                    rng = (range(len(banks)) if w == 512
                           else range(len(banks) - 1, -1, -1))
                    for b in rng:
                        if banks[b] + w <= 512:
                            sg["pos"] = b * 512 + banks[b]
                            banks[b] += w
                            placed = sg
                            break
                    if placed:
                        break
                if placed is None:
                    groups.append(cur)
                    new_group()
                    continue
                if first:
                    placed["first"], first = True, False
                rem.remove(placed)
                cur.append(placed)
                blk.append(placed)
            blk[-1]["last"] = True
    if cur:
        groups.append(cur)

    # engine routing + roles for the l accumulation
    for blk in blocks.values():
        total = sum(s["w"] for s in blk)
        gp_target = gp_frac * total if total >= gp_min_cols else 0
        gp_cols = 0
        blk[0]["eng"] = "dve"
        for sg in blk[1:]:
            if sg["w"] == 512 and gp_cols < gp_target:
                sg["eng"] = "gp"
                gp_cols += sg["w"]
            else:
                sg["eng"] = "dve"
        for eng in ("dve", "gp"):
            es = [s for s in blk if s["eng"] == eng]
            if not es:
                continue
            if len(es) >= 2 and es[0]["w"] == 512 and es[1]["w"] == 512:
                es[0]["role"], es[1]["role"] = "pair1", "pair2"
                rest = es[2:]
            else:
                es[0]["role"] = "copy"
                rest = es[1:]
            for s in rest:
                s["role"] = "add"
    return groups


def _build(n_heads=HPC, la=3, p_bufs=10):
    nc = bacc.Bacc("TRN2", target_bir_lowering=False, debug=False,
                   num_devices=N_CORES)
    qt = nc.dram_tensor("qt", [n_heads, 128, S], BF16, kind="ExternalInput")
    kt = nc.dram_tensor("kt", [n_heads, 128, S], BF16, kind="ExternalInput")
    v = nc.dram_tensor("v", [n_heads, 128, NKT, D], F32R, kind="ExternalInput")
    # tri[r, c] = 1 where c >= r else 0 (causal keep-triangle)
    tri = nc.dram_tensor("tri", [128, 128], F32R, kind="ExternalInput")
    ones = nc.dram_tensor("ones", [128, 128], F32R, kind="ExternalInput")
    zeros = nc.dram_tensor("zeros", [128, 384], F32R, kind="ExternalInput")
    out = nc.dram_tensor("out", [n_heads, 128, S], BF16, kind="ExternalOutput")

    groups = _plan(n_heads)

    with tile.TileContext(nc) as tc:
        with (tc.tile_pool(name="heads", bufs=2) as hp,
              tc.tile_pool(name="consts", bufs=1) as cp,
              tc.tile_pool(name="pp", bufs=p_bufs) as pp,
              tc.tile_pool(name="l2p", bufs=4) as l2p,
              tc.tile_pool(name="outp", bufs=3) as outp,
              tc.tile_pool(name="ps_s", bufs=1, space="PSUM") as ps_s,
              tc.tile_pool(name="ps_l", bufs=1, space="PSUM") as ps_l,
              tc.tile_pool(name="ps_c", bufs=2, space="PSUM") as ps_c):
            tri_sb = cp.tile([128, 128], F32R)
            nc.sync.dma_start(tri_sb, tri[:, :])
            ones_sb = cp.tile([128, 128], F32R)
            nc.sync.dma_start(ones_sb, ones[:, :])
            zeros_sb = cp.tile([128, 384], F32R)
            nc.sync.dma_start(zeros_sb, zeros[:, :])

            head_sb = {}     # h -> (qt_sb, kt_sb, v_sb)
            blk_state = {}   # (h, j) -> dict(ctx_ps, accumulators, stashes)
            cleanups = []    # (due_group_idx, state dict) pending PE l-reduce
            epilogues = []   # (due_group_idx, state dict) pending recip/mult

            def prep_head(h):
                if h in head_sb:
                    return head_sb[h]
                qt_sb = hp.tile([128, S], BF16, tag="qt", name="qt_sb")
                kt_sb = hp.tile([128, S], BF16, tag="kt", name="kt_sb")
                v_sb = hp.tile([128, NKT, D], F32R, tag="v", name="v_sb")
                for c0 in range(0, S, 512):
                    nc.sync.dma_start(kt_sb[:, c0:c0 + 512], kt[h, :, c0:c0 + 512])
                    nc.sync.dma_start(qt_sb[:, c0:c0 + 512], qt[h, :, c0:c0 + 512])
                for t0 in range(0, NKT, 4):
                    nc.sync.dma_start(v_sb[:, t0:t0 + 4, :], v[h, :, t0:t0 + 4, :])
                head_sb[h] = (qt_sb, kt_sb, v_sb)
                return head_sb[h]

            def emit_s(gi, grp):
                if gi % 2 == 0:
                    s_ps = ps_s.tile([128, 1536], F32, tag="sA", name="s_psA")
                else:
                    s_ps = ps_s.tile([128, 1024], F32, tag="sB", name="s_psB")
                for sg in grp:
                    qt_sb, kt_sb, _ = prep_head(sg["h"])
                    t, j = sg["t"], sg["j"]
                    nc.tensor.matmul(
                        s_ps[:, sg["pos"]:sg["pos"] + sg["w"]],
                        kt_sb[:, t * 128:(t + 1) * 128],
                        qt_sb[:, j * QB + sg["o"]:(j + 1) * QB],
                        start=True, stop=True)
                return s_ps

            def flush_cleanups(i, force=False):
                # PE partition-reduce of the l accumulators into the single l
                # PSUM bank, one group after the block's last segment (so the
                # trailing DVE/gpsimd adds are done by the time PE gets here).
                while cleanups and (force or cleanups[0][0] <= i):
                    _, st = cleanups.pop(0)
                    st["l_ps"] = ps_l.tile([128, QB], F32, tag="l",
                                           name="l_ps")
                    accs = [a for a in (st.get("l2_dve"), st.get("l2_gp"))
                            if a is not None]
                    for ai, acc in enumerate(accs):
                        nc.tensor.matmul(st["l_ps"][:, :], ones_sb, acc[:, :],
                                         start=(ai == 0),
                                         stop=(ai == len(accs) - 1))
                    epilogues.append((st.pop("due") + 2, st))

            def flush_epilogues(i, force=False):
                while epilogues and (force or epilogues[0][0] <= i):
                    _, st = epilogues.pop(0)
                    h, j = st["h"], st["j"]
                    recip_sb = outp.tile([128, QB], F32, tag="recip",
                                         name="recip_sb")
                    nc.vector.reciprocal_approx_fast(recip_sb, st["l_ps"])
                    ctx_sb = outp.tile([128, QB], BF16, tag="ctx_out",
                                       name="ctx_sb")
                    nc.vector.tensor_tensor(out=ctx_sb, in0=st["ctx_ps"][:, :],
                                            in1=recip_sb, op=MULT)
                    nc.sync.dma_start(out[h, :, j * QB:(j + 1) * QB], ctx_sb)

            pending = [emit_s(gi, g) for gi, g in enumerate(groups[:la])]
            for i, grp in enumerate(groups):
                if i + la < len(groups):
                    pending.append(emit_s(i + la, groups[i + la]))
                s_ps = pending.pop(0)
                flush_cleanups(i)
                flush_epilogues(i)

                x1 = max(sg["pos"] + sg["w"] for sg in grp)
                p_sb = pp.tile([128, 1536], F32R, tag="p", name="p_sb")
                nc.scalar.activation(p_sb[:, :x1], s_ps[:, :x1], EXP,
                                     scale=SCALE)

                # zero the below-diagonal triangles, two per DVE op
                diag = sorted((sg["pos"] for sg in grp
                               if sg["t"] >= 4 * sg["j"]))
                while diag:
                    if len(diag) >= 2:
                        p0, p1 = diag.pop(0), diag.pop(0)
                        pap = bass.AP(tensor=p_sb.tensor,
                                      offset=p_sb.offset + p0,
                                      ap=[p_sb.ap[0], [p1 - p0, 2], [1, 128]])
                        tap = bass.AP(tensor=tri_sb.tensor,
                                      offset=tri_sb.offset,
                                      ap=[tri_sb.ap[0], [0, 2], [1, 128]])
                        nc.vector.tensor_tensor(out=pap, in0=pap, in1=tap,
                                                op=MULT)
                    else:
                        p0 = diag.pop(0)
                        nc.vector.tensor_tensor(
                            out=p_sb[:, p0:p0 + 128],
                            in0=p_sb[:, p0:p0 + 128],
                            in1=tri_sb, op=MULT)

                for sg in grp:
                    h, j, t = sg["h"], sg["j"], sg["t"]
                    _, _, v_sb = head_sb[h]
                    if sg["first"]:
                        blk_state[(h, j)] = dict(
                            ctx_ps=ps_c.tile([128, QB], F32, tag="ctx",
                                             name="ctx_ps"))
                    st = blk_state[(h, j)]
                    nc.tensor.matmul(
                        st["ctx_ps"][:, sg["o"]:], v_sb[:, t, :],
                        p_sb[:, sg["pos"]:sg["pos"] + sg["w"]],
                        start=sg["first"], stop=sg["last"])

                for sg in grp:
                    h, j = sg["h"], sg["j"]
                    st = blk_state[(h, j)]
                    psrc = p_sb[:, sg["pos"]:sg["pos"] + sg["w"]]
                    eng = nc.gpsimd if sg["eng"] == "gp" else nc.vector
                    akey = "l2_" + sg["eng"]
                    role = sg["role"]
                    if role == "pair1":
                        st["stash_" + sg["eng"]] = psrc
                    elif role == "pair2":
                        acc = st[akey] = l2p.tile([128, QB], F32R,
                                                  tag=akey, name=akey)
                        eng.tensor_tensor(out=acc[:, :],
                                          in0=st.pop("stash_" + sg["eng"]),
                                          in1=psrc, op=ADD)
                    elif role == "copy":
                        acc = st[akey] = l2p.tile([128, QB], F32R,
                                                  tag=akey, name=akey)
                        eng.tensor_copy(acc[:, sg["o"]:], psrc)
                        if sg["o"]:
                            eng.tensor_copy(acc[:, :sg["o"]],
                                            zeros_sb[:, :sg["o"]])
                    else:
                        acc = st[akey]
                        eng.tensor_tensor(out=acc[:, sg["o"]:],
                                          in0=acc[:, sg["o"]:],
                                          in1=psrc, op=ADD)
                    if sg["last"]:           # block end
                        st["h"], st["j"], st["due"] = h, j, i
                        cleanups.append((i + 1, st))
                        del blk_state[(h, j)]

            flush_cleanups(0, force=True)
            flush_epilogues(0, force=True)

    nc.compile()
    return nc


_NC_CACHE = None


def _get_nc():
    global _NC_CACHE
    if _NC_CACHE is None:
        _NC_CACHE = _build()
    return _NC_CACHE


def _prep_inputs(q, k, v):
    """Full [b,h,s,d] f32 inputs -> per-core input maps (q/k bf16, v f32)."""
    bf = ml_dtypes.bfloat16
    qf = np.asarray(q, np.float32).reshape(B * H, S, D)
    kf = np.asarray(k, np.float32).reshape(B * H, S, D)
    vf = np.asarray(v, np.float32).reshape(B * H, S, D)
    qt = qf.transpose(0, 2, 1).astype(bf)                    # [64, d, s]
    kt = kf.transpose(0, 2, 1).astype(bf)
    vr = np.ascontiguousarray(
        vf.reshape(B * H, NKT, 128, D).transpose(0, 2, 1, 3))
    tri_np = (np.arange(128)[None, :] >= np.arange(128)[:, None]).astype(np.float32)
    ones_np = np.ones((128, 128), dtype=np.float32)
    zeros_np = np.zeros((128, 384), dtype=np.float32)
    in_maps = []
    for c in range(N_CORES):
        sl = slice(c * HPC, (c + 1) * HPC)
        in_maps.append({
            "qt": np.ascontiguousarray(qt[sl]),
            "kt": np.ascontiguousarray(kt[sl]),
            "v": vr[sl],
            "tri": tri_np,
            "ones": ones_np,
            "zeros": zeros_np,
        })
    return in_maps


def kernel(query_layer, key_layer, value_layer, attention_mask):
    """Full-input causal attention; returns [b, s, h*d] float32."""
    # attention_mask is the standard causal mask (True = masked); the kernel
    # hardcodes causal masking, so the mask tensor itself is not shipped.
    in_maps = _prep_inputs(query_layer, key_layer, value_layer)
    nc = _get_nc()
    res = run_bass_kernel_spmd(nc, in_maps, core_ids=list(range(N_CORES)))

    # [64(bh), d, s] bf16 -> out[b, s, h*D+d] f32 in a single transpose pass
    o_all = np.concatenate([res.results[c]["out"] for c in range(N_CORES)],
                           axis=0)
    return np.ascontiguousarray(
        o_all.astype(np.float32).reshape(B, H, D, S).transpose(0, 3, 1, 2)
    ).reshape(B, S, H * D)


# revision 28
# speedup vs baseline: 1.5551x; 1.5551x over previous
"""Causal multi-head attention (b=2, h=32, s=2048, d=128, fp32) on 8 TRN2 NeuronCores.

Sharding: the 64 (batch, head) pairs are split 8-per-core (tensor parallel over
heads); each core runs an identical Bass/Tile kernel on its own heads.

Per-core kernel, S^T formulation:
  Q^T/K^T ship as bf16 (halves their DMA; bf16 matmuls run 1 cycle/row at any
  width).  The 40 causal S^T tile-segments per head are packed (widest-fit-
  decreasing, PSUM-bank-aligned) into groups alternating 1536/1024 columns
  (tags sA/sB: 3+2 banks), so each exp instruction covers ~1280 columns and
  the scalar engine's ~285ns/instruction overhead is amortized over ~109
  instructions.  P^T = exp(S^T/sqrt(d)) is written as f32r (bf16 activation
  output measures ~25% slower per column), no max-subtraction needed.
  Diagonal-tile triangles are zeroed by DVE multiplies with a constant 0/1
  triangle, two triangles fused per op via a strided access pattern.
  ctx^T[d,q] accumulates in PSUM via V-stationary f32r matmuls (2 x 1 bank).
  The softmax denominator l[q] is accumulated per 128-k-partition in two
  independent SBUF accumulators - one summed by the DVE, one by the gpsimd
  engine (decoupled serial chains, no cross-engine hops) - then both are
  partition-reduced by ones-stationary PE matmuls into a single PSUM bank
  (3+2+1+2 = 8 banks total).  Epilogue: recip (DVE) + ctx*recip (DVE), DMA
  out as bf16.  Cleanups/epilogues trail the main loop by 1-2 groups so no
  engine head-of-line blocks another.
"""
import math
import sys

if '/opt/trn_rl_repo' not in sys.path:
    sys.path.insert(0, '/opt/trn_rl_repo')

import numpy as np
import ml_dtypes

import concourse.bass as bass
import concourse.tile as tile
from concourse import mybir, bacc
from concourse.bass_utils import run_bass_kernel_spmd

F32 = mybir.dt.float32
F32R = mybir.dt.float32r
BF16 = mybir.dt.bfloat16
EXP = mybir.ActivationFunctionType.Exp
MULT = mybir.AluOpType.mult
ADD = mybir.AluOpType.add

B, H, S, D = 2, 32, 2048, 128
N_CORES = 8
HPC = (B * H) // N_CORES     # (b,h) pairs per core
QB = 512                     # q-block width
NQB = S // QB
NKT = S // 128               # k-tiles per head
SCALE = 1.0 / math.sqrt(D)


def _plan(n_heads, pe_js=(2,)):
    """Pack all (head, q-block, k-tile) S^T segments into PSUM groups with
    capacities alternating 1536/1024 columns (widest-fit-decreasing inside a
    block; 512-wide segs front-fill banks, narrower back-fill).  Each segment
    gets: pos (column in group), eng ('dve'|'gp') and role
    ('pair1'|'pair2'|'copy'|'add') for the l accumulation."""
    groups = []
    cur, banks = [], [0] * 3
    blocks = {}

    def new_group():
        nonlocal cur, banks
        nbanks = 3 if len(groups) % 2 == 0 else 2
        cur, banks = [], [0] * nbanks

    new_group()
    for h in range(n_heads):
        for j in range(NQB):
            rem = []
            for t in range(4 * j + 4):    # causal: k-tiles 0..4j+3
                o = max(t - 4 * j, 0) * 128
                rem.append(dict(h=h, j=j, t=t, w=QB - o, o=o,
                                first=False, last=False))
            blk = blocks[(h, j)] = []
            first = True
            while rem:
                placed = None
                for sg in sorted(rem, key=lambda s: -s["w"]):
                    w = sg["w"]
                    rng = (range(len(banks)) if w == 512
                           else range(len(banks) - 1, -1, -1))
                    for b in rng:
                        if banks[b] + w <= 512:
                            sg["pos"] = b * 512 + banks[b]
                            banks[b] += w
                            placed = sg
                            break
                    if placed:
                        break
                if placed is None:
                    groups.append(cur)
                    new_group()
                    continue
                if first:
                    placed["first"], first = True, False
                rem.remove(placed)
                cur.append(placed)
                blk.append(placed)
            blk[-1]["last"] = True
    if cur:
        groups.append(cur)

    # l-accumulation routing: blocks in pe_js sum l on the tensor engine
    # (deferred ones-matmul batch straight into PSUM); the rest accumulate
    # on the DVE with a pair-add init when the first two segs are 512-wide.
    for (h, j), blk in blocks.items():
        if j in pe_js:
            for s in blk:
                s["role"] = "pe"
            continue
        if len(blk) >= 2 and blk[0]["w"] == 512 and blk[1]["w"] == 512:
            blk[0]["role"], blk[1]["role"] = "pair1", "pair2"
            rest = blk[2:]
        else:
            blk[0]["role"] = "copy"
            rest = blk[1:]
        for s in rest:
            s["role"] = "add"
    return groups


def _build(n_heads=HPC, la=3, p_bufs=10):
    nc = bacc.Bacc("TRN2", target_bir_lowering=False, debug=False,
                   num_devices=N_CORES)
    qt = nc.dram_tensor("qt", [n_heads, 128, S], BF16, kind="ExternalInput")
    kt = nc.dram_tensor("kt", [n_heads, 128, S], BF16, kind="ExternalInput")
    v = nc.dram_tensor("v", [n_heads, 128, NKT, D], F32R, kind="ExternalInput")
    # tri[r, c] = 1 where c >= r else 0 (causal keep-triangle)
    tri = nc.dram_tensor("tri", [128, 128], F32R, kind="ExternalInput")
    ones = nc.dram_tensor("ones", [128, 128], F32R, kind="ExternalInput")
    zeros = nc.dram_tensor("zeros", [128, 384], F32R, kind="ExternalInput")
    out = nc.dram_tensor("out", [n_heads, 128, S], BF16, kind="ExternalOutput")

    groups = _plan(n_heads)

    with tile.TileContext(nc) as tc:
        with (tc.tile_pool(name="heads", bufs=2) as hp,
              tc.tile_pool(name="consts", bufs=1) as cp,
              tc.tile_pool(name="pp", bufs=p_bufs) as pp,
              tc.tile_pool(name="l2p", bufs=4) as l2p,
              tc.tile_pool(name="outp", bufs=3) as outp,
              tc.tile_pool(name="ps_s", bufs=1, space="PSUM") as ps_s,
              tc.tile_pool(name="ps_l", bufs=1, space="PSUM") as ps_l,
              tc.tile_pool(name="ps_c", bufs=2, space="PSUM") as ps_c):
            tri_sb = cp.tile([128, 128], F32R)
            nc.sync.dma_start(tri_sb, tri[:, :])
            ones_sb = cp.tile([128, 128], F32R)
            nc.sync.dma_start(ones_sb, ones[:, :])
            zeros_sb = cp.tile([128, 384], F32R)
            nc.sync.dma_start(zeros_sb, zeros[:, :])

            head_sb = {}     # h -> (qt_sb, kt_sb, v_sb)
            blk_state = {}   # (h, j) -> dict(ctx_ps, accumulators, stashes)
            cleanups = []    # (due_group_idx, state dict) pending PE l-reduce
            epilogues = []   # (due_group_idx, state dict) pending recip/mult

            def prep_head(h):
                if h in head_sb:
                    return head_sb[h]
                qt_sb = hp.tile([128, S], BF16, tag="qt", name="qt_sb")
                kt_sb = hp.tile([128, S], BF16, tag="kt", name="kt_sb")
                v_sb = hp.tile([128, NKT, D], F32R, tag="v", name="v_sb")
                for c0 in range(0, S, 512):
                    nc.sync.dma_start(kt_sb[:, c0:c0 + 512], kt[h, :, c0:c0 + 512])
                    nc.sync.dma_start(qt_sb[:, c0:c0 + 512], qt[h, :, c0:c0 + 512])
                for t0 in range(0, NKT, 4):
                    nc.sync.dma_start(v_sb[:, t0:t0 + 4, :], v[h, :, t0:t0 + 4, :])
                head_sb[h] = (qt_sb, kt_sb, v_sb)
                return head_sb[h]

            def emit_s(gi, grp):
                if gi % 2 == 0:
                    s_ps = ps_s.tile([128, 1536], F32, tag="sA", name="s_psA")
                else:
                    s_ps = ps_s.tile([128, 1024], F32, tag="sB", name="s_psB")
                for sg in grp:
                    qt_sb, kt_sb, _ = prep_head(sg["h"])
                    t, j = sg["t"], sg["j"]
                    nc.tensor.matmul(
                        s_ps[:, sg["pos"]:sg["pos"] + sg["w"]],
                        kt_sb[:, t * 128:(t + 1) * 128],
                        qt_sb[:, j * QB + sg["o"]:(j + 1) * QB],
                        start=True, stop=True)
                return s_ps

            def flush_cleanups(i, force=False):
                # PE partition-reduce into the single l PSUM bank, one group
                # after the block's last segment: either one ones-matmul over
                # the DVE-accumulated l2, or (PE blocks) a deferred batch of
                # ones-matmuls over the block's still-live P segments.
                while cleanups and (force or cleanups[0][0] <= i):
                    _, st = cleanups.pop(0)
                    st["l_ps"] = ps_l.tile([128, QB], F32, tag="l",
                                           name="l_ps")
                    if "l2_dve" in st:
                        nc.tensor.matmul(st["l_ps"][:, :], ones_sb,
                                         st["l2_dve"][:, :],
                                         start=True, stop=True)
                    else:
                        psegs = st.pop("psegs")
                        for si, (psrc, o) in enumerate(psegs):
                            nc.tensor.matmul(st["l_ps"][:, o:], ones_sb[:, :],
                                             psrc, start=(si == 0),
                                             stop=(si == len(psegs) - 1))
                    epilogues.append((st.pop("due") + 2, st))

            def flush_epilogues(i, force=False):
                while epilogues and (force or epilogues[0][0] <= i):
                    _, st = epilogues.pop(0)
                    h, j = st["h"], st["j"]
                    recip_sb = outp.tile([128, QB], F32, tag="recip",
                                         name="recip_sb")
                    nc.vector.reciprocal_approx_fast(recip_sb, st["l_ps"])
                    ctx_sb = outp.tile([128, QB], BF16, tag="ctx_out",
                                       name="ctx_sb")
                    nc.vector.tensor_tensor(out=ctx_sb, in0=st["ctx_ps"][:, :],
                                            in1=recip_sb, op=MULT)
                    nc.sync.dma_start(out[h, :, j * QB:(j + 1) * QB], ctx_sb)

            pending = [emit_s(gi, g) for gi, g in enumerate(groups[:la])]
            for i, grp in enumerate(groups):
                if i + la < len(groups):
                    pending.append(emit_s(i + la, groups[i + la]))
                s_ps = pending.pop(0)
                flush_cleanups(i)
                flush_epilogues(i)

                x1 = max(sg["pos"] + sg["w"] for sg in grp)
                p_sb = pp.tile([128, 1536], F32R, tag="p", name="p_sb")
                nc.scalar.activation(p_sb[:, :x1], s_ps[:, :x1], EXP,
                                     scale=SCALE)

                # zero the below-diagonal triangles, two per DVE op
                diag = sorted((sg["pos"] for sg in grp
                               if sg["t"] >= 4 * sg["j"]))
                while diag:
                    if len(diag) >= 2:
                        p0, p1 = diag.pop(0), diag.pop(0)
                        pap = bass.AP(tensor=p_sb.tensor,
                                      offset=p_sb.offset + p0,
                                      ap=[p_sb.ap[0], [p1 - p0, 2], [1, 128]])
                        tap = bass.AP(tensor=tri_sb.tensor,
                                      offset=tri_sb.offset,
                                      ap=[tri_sb.ap[0], [0, 2], [1, 128]])
                        nc.vector.tensor_tensor(out=pap, in0=pap, in1=tap,
                                                op=MULT)
                    else:
                        p0 = diag.pop(0)
                        nc.vector.tensor_tensor(
                            out=p_sb[:, p0:p0 + 128],
                            in0=p_sb[:, p0:p0 + 128],
                            in1=tri_sb, op=MULT)

                for sg in grp:
                    h, j, t = sg["h"], sg["j"], sg["t"]
                    _, _, v_sb = head_sb[h]
                    if sg["first"]:
                        blk_state[(h, j)] = dict(
                            ctx_ps=ps_c.tile([128, QB], F32, tag="ctx",
                                             name="ctx_ps"))
                    st = blk_state[(h, j)]
                    nc.tensor.matmul(
                        st["ctx_ps"][:, sg["o"]:], v_sb[:, t, :],
                        p_sb[:, sg["pos"]:sg["pos"] + sg["w"]],
                        start=sg["first"], stop=sg["last"])

                for sg in grp:
                    h, j = sg["h"], sg["j"]
                    st = blk_state[(h, j)]
                    psrc = p_sb[:, sg["pos"]:sg["pos"] + sg["w"]]
                    role = sg["role"]
                    if role == "pe":
                        st.setdefault("psegs", []).append((psrc, sg["o"]))
                    elif role == "pair1":
                        st["stash"] = psrc
                    elif role == "pair2":
                        acc = st["l2_dve"] = l2p.tile([128, QB], F32R,
                                                      tag="l2", name="l2_sb")
                        nc.vector.tensor_tensor(out=acc[:, :],
                                                in0=st.pop("stash"),
                                                in1=psrc, op=ADD)
                    elif role == "copy":
                        acc = st["l2_dve"] = l2p.tile([128, QB], F32R,
                                                      tag="l2", name="l2_sb")
                        nc.vector.tensor_copy(acc[:, sg["o"]:], psrc)
                        if sg["o"]:
                            nc.vector.tensor_copy(acc[:, :sg["o"]],
                                                  zeros_sb[:, :sg["o"]])
                    else:
                        acc = st["l2_dve"]
                        nc.vector.tensor_tensor(out=acc[:, sg["o"]:],
                                                in0=acc[:, sg["o"]:],
                                                in1=psrc, op=ADD)
                    if sg["last"]:           # block end
                        st["h"], st["j"], st["due"] = h, j, i
                        cleanups.append((i + 1, st))
                        del blk_state[(h, j)]

            flush_cleanups(0, force=True)
            flush_epilogues(0, force=True)

    nc.compile()
    return nc


_NC_CACHE = None


def _get_nc():
    global _NC_CACHE
    if _NC_CACHE is None:
        _NC_CACHE = _build()
    return _NC_CACHE


def _prep_inputs(q, k, v):
    """Full [b,h,s,d] f32 inputs -> per-core input maps (q/k bf16, v f32)."""
    bf = ml_dtypes.bfloat16
    qf = np.asarray(q, np.float32).reshape(B * H, S, D)
    kf = np.asarray(k, np.float32).reshape(B * H, S, D)
    vf = np.asarray(v, np.float32).reshape(B * H, S, D)
    qt = qf.transpose(0, 2, 1).astype(bf)                    # [64, d, s]
    kt = kf.transpose(0, 2, 1).astype(bf)
    vr = np.ascontiguousarray(
        vf.reshape(B * H, NKT, 128, D).transpose(0, 2, 1, 3))
    tri_np = (np.arange(128)[None, :] >= np.arange(128)[:, None]).astype(np.float32)
    ones_np = np.ones((128, 128), dtype=np.float32)
    zeros_np = np.zeros((128, 384), dtype=np.float32)
    in_maps = []
    for c in range(N_CORES):
        sl = slice(c * HPC, (c + 1) * HPC)
        in_maps.append({
            "qt": np.ascontiguousarray(qt[sl]),
            "kt": np.ascontiguousarray(kt[sl]),
            "v": vr[sl],
            "tri": tri_np,
            "ones": ones_np,
            "zeros": zeros_np,
        })
    return in_maps


def kernel(query_layer, key_layer, value_layer, attention_mask):
    """Full-input causal attention; returns [b, s, h*d] float32."""
    # attention_mask is the standard causal mask (True = masked); the kernel
    # hardcodes causal masking, so the mask tensor itself is not shipped.
    in_maps = _prep_inputs(query_layer, key_layer, value_layer)
    nc = _get_nc()
    res = run_bass_kernel_spmd(nc, in_maps, core_ids=list(range(N_CORES)))

    # [64(bh), d, s] bf16 -> out[b, s, h*D+d] f32 in a single transpose pass
    o_all = np.concatenate([res.results[c]["out"] for c in range(N_CORES)],
                           axis=0)
    return np.ascontiguousarray(
        o_all.astype(np.float32).reshape(B, H, D, S).transpose(0, 3, 1, 2)
    ).reshape(B, S, H * D)
